# revision 1
# baseline (speedup 1.0000x reference)
import numpy as np
import jax
import jax.numpy as jnp
from functools import partial

# nn_LocalAttention: B,C,T,H,W = 8,64,4,56,56; data-parallel over B across 8 cores.
K = 3
PAD = 1
HID = 64
EPS = 1e-5
B, C, T, H, W = 8, 64, 4, 56, 56
N_CORES = 8


def _unfold(x, H, W):
    xp = jnp.pad(x, ((0, 0), (0, 0), (PAD, PAD), (PAD, PAD)))
    patches = jnp.stack([xp[:, :, i:i + H, j:j + W]
                         for i in range(K) for j in range(K)], axis=2)
    Bl, CT = x.shape[:2]
    return patches.reshape(Bl, CT, K * K, H * W)


def _fold(y, H, W):
    Bl, CT = y.shape[:2]
    y = y.reshape(Bl, CT, K * K, H, W)
    out = jnp.zeros((Bl, CT, H + 2 * PAD, W + 2 * PAD), y.dtype)
    idx = 0
    for i in range(K):
        for j in range(K):
            out = out.at[:, :, i:i + H, j:j + W].add(y[:, :, idx])
            idx += 1
    return out[:, :, PAD:PAD + H, PAD:PAD + W]


@partial(jax.pmap, axis_name='b')
def _run(x, w_in, b_in, w_out, b_out, gamma, beta):
    # x: (Bl, C, T, H, W) local batch shard
    Bl = x.shape[0]
    k2 = K * K
    h = jnp.einsum('oc,bcthw->bothw', w_in, x) + b_in[None, :, None, None, None]
    theta, phi, g = jnp.split(h, 3, axis=1)

    def unf(z):
        z = z.reshape(Bl, HID * T, H, W)
        return _unfold(z, H, W).reshape(Bl, HID, T * k2, H * W)

    tu, pu, gu = unf(theta), unf(phi), unf(g)
    attn = jnp.einsum('bcts,bcps->btps', tu, pu) / (T * k2)
    out = jnp.einsum('btgs,bcgs->bcts', attn, gu)
    out = out.reshape(Bl, HID * T, k2, H * W)
    out = _fold(out, H, W).reshape(Bl, HID, T, H, W)
    out = jnp.einsum('oc,bcthw->bothw', w_out, out) + b_out[None, :, None, None, None]
    # BatchNorm3d training-mode stats across the GLOBAL batch: allreduce moments
    mu = jax.lax.pmean(out.mean(axis=(0, 2, 3, 4)), 'b')
    m2 = jax.lax.pmean((out * out).mean(axis=(0, 2, 3, 4)), 'b')
    var = m2 - mu * mu
    mu = mu[None, :, None, None, None]
    var = var[None, :, None, None, None]
    out = (out - mu) * jax.lax.rsqrt(var + EPS)
    out = out * gamma[None, :, None, None, None] + beta[None, :, None, None, None]
    return x + out


def kernel(**inputs):
    x = np.asarray(inputs['x'], dtype=np.float32)
    shard = B // N_CORES
    xs = x.reshape(N_CORES, shard, C, T, H, W)

    def rep(name):
        a = np.asarray(inputs[name], dtype=np.float32)
        return np.broadcast_to(a, (N_CORES,) + a.shape)

    out = _run(xs, rep('w_in'), rep('b_in'), rep('w_out'), rep('b_out'),
               rep('gamma'), rep('beta'))
    return np.asarray(out).reshape(B, C, T, H, W).astype(np.float32)


# revision 2
# speedup vs baseline: 1.3366x; 1.3366x over previous
import numpy as np
import jax
import jax.numpy as jnp
from functools import partial

# nn_LocalAttention via row-block bipartite matmuls + precomputed boundary mask.
# Exact reformulation (validated): unfold/attention/fold collapse into two big
# batched matmuls per row-block against a masked bipartite logit tensor.
K = 3
PAD = 1
HID = 64
EPS = 1e-5
B, C, T, H, W = 8, 64, 4, 56, 56
N_CORES = 8
BW = 28           # query column-block width
NB = W // BW      # 2 blocks per row
V = BW + 4        # key window width (±2 halo)
k2 = K * K


def _build_mask():
    def n1_table(L):
        t = np.zeros((L, 5), np.float32)
        for pos in range(L):
            for d in range(-2, 3):
                n = 0
                for d1 in (-1, 0, 1):
                    for d2 in (-1, 0, 1):
                        if d2 - d1 == d and 0 <= pos - d1 < L:
                            n += 1
                t[pos, d + 2] = n
        return t

    n1h, n1w = n1_table(H), n1_table(W)
    M = np.zeros((H, NB, BW, 5, V), np.float32)
    hh = np.arange(H)
    for s in range(NB):
        for w in range(BW):
            wg = s * BW + w
            for r in range(5):
                zh = hh + r - 2
                okh = (zh >= 0) & (zh < H)
                for v in range(V):
                    zw = s * BW - 2 + v
                    uv = zw - wg
                    if abs(uv) > 2 or not (0 <= zw < W):
                        continue
                    M[:, s, w, r, v] = okh * n1h[:, r] * n1w[wg, uv + 2] / (T * k2)
    return M


_MASK = jnp.asarray(_build_mask())


@partial(jax.pmap, axis_name='b')
def _run(x, w_in, b_in, w_out, b_out, gamma, beta):
    Bl = x.shape[0]
    h = jnp.einsum('oc,bcthw->bothw', w_in, x) + b_in[None, :, None, None, None]
    theta, phi, g = jnp.split(h, 3, axis=1)

    # key windows: pad rows/cols by 2, slice 5-row bands per query row, col blocks
    def windows(z):
        zp = jnp.pad(z, ((0, 0), (0, 0), (0, 0), (2, 2), (2, 2)))
        rows = jnp.stack([zp[:, :, :, r:r + H, :] for r in range(5)], axis=3)
        cols = jnp.stack([rows[:, :, :, :, :, s * BW:s * BW + V]
                          for s in range(NB)], axis=5)
        return cols  # (Bl, C', T, 5, H, NB, V)

    pw, gw = windows(phi), windows(g)
    thb = theta.reshape(Bl, HID, T, H, NB, BW)
    A = jnp.einsum('bcthsw,bcprhsv->bhstwprv', thb, pw)
    A = A * _MASK[None, :, :, None, :, None, :, :]
    F = jnp.einsum('bhstwprv,bcprhsv->bcthsw', A, gw)
    out = F.reshape(Bl, HID, T, H, W)
    out = jnp.einsum('oc,bcthw->bothw', w_out, out) + b_out[None, :, None, None, None]
    mu = jax.lax.pmean(out.mean(axis=(0, 2, 3, 4)), 'b')
    m2 = jax.lax.pmean((out * out).mean(axis=(0, 2, 3, 4)), 'b')
    var = m2 - mu * mu
    out = (out - mu[None, :, None, None, None]) * jax.lax.rsqrt(var[None, :, None, None, None] + EPS)
    out = out * gamma[None, :, None, None, None] + beta[None, :, None, None, None]
    return x + out


def kernel(**inputs):
    x = np.asarray(inputs['x'], dtype=np.float32)
    shard = B // N_CORES
    xs = x.reshape(N_CORES, shard, C, T, H, W)

    def rep(name):
        a = np.asarray(inputs[name], dtype=np.float32)
        return np.broadcast_to(a, (N_CORES,) + a.shape)

    out = _run(xs, rep('w_in'), rep('b_in'), rep('w_out'), rep('b_out'),
               rep('gamma'), rep('beta'))
    return np.asarray(out).reshape(B, C, T, H, W).astype(np.float32)
